# revision 1
# baseline (speedup 1.0000x reference)
"""HMLDM loss kernel for 8x Trainium2 NeuronCores.

Math (see reference):
  z = softmax(latent_z, 1); w = softmax(latent_w, 1)
  dist[i,j] = ||z_i - w_j||;  val = exp(-(dist+EPS))
  z1 = sum_ij exp(gr_i) * val[i,j] * exp(gc_j)
  z2 = sum_e w_e * (gr[r_e] + gc[c_e] - dist(z[r_e], w[c_e]))
  out = z1 - z2

Sharding: rows (N) split across 8 cores; latent_w/gamma_cols replicated.
Edges split by position; per-edge raw rows are host-joined (pure np.take of
inputs) into each core's edge partition.

Device algorithm per core (c = core id, NL = N/8 = 2048 local rows):
  phase 0: build bf16 transposed matmul operand tables via DRAM roundtrip
           (pad to 128 cols for the xbar DMA transpose); exp(gammas).
  main: for each i-block (512 local rows) over 64 j-tiles (128 cols each):
    PE aug-matmul K=10: sq[j,i] = w2_j + z2_i - 2 w_j.z_i   (PSUM f32)
    DVE: dist_in = max(sq, 0) -> bf16 SBUF  (bf16 rounding can make sq<0)
    ACT: dist = sqrt(dist_in)  [sqrt table set, one op per i-block]
    ACT: val = exp(-dist - EPS)  [exp set]
    PE reduce-matmul: colsum[i] += ec_j^T val[j,i]  (lhsT = exp(gc) column)
  edges (8 chunks, 2 per i-block, riding the ACT table-set phases):
    ez = exp(z_raw), ew = exp(w_raw)  (unnormalized softmax numerators)
    sz, sw row sums; zz = sum ez^2, ww = sum ew^2, zw = sum ez*ew
    sq_e = zz/sz^2 + ww/sw^2 - 2 zw/(sz*sw); clamp; dist_e = sqrt
    z2 partial += sum w_e * (gr_e + gc_e - dist_e)
  final: z1 = sum_i exp(gr_i) * colsum[i]; out = z1 - z2
"""
import numpy as np
import ml_dtypes
from contextlib import ExitStack

import concourse.bass as bass
import concourse.bacc as bacc
import concourse.tile as tile
import concourse.mybir as mybir
from concourse.bass_utils import run_bass_kernel_spmd

F32 = mybir.dt.float32
BF16 = mybir.dt.bfloat16
I32 = mybir.dt.int32
AF = mybir.ActivationFunctionType
ALU = mybir.AluOpType
AX = mybir.AxisListType

N, M, D, E = 16384, 8192, 8, 2_000_000
EPS = 1e-6
NCORES = 8
NL = N // NCORES          # 2048 local rows
IB = 512                  # i-block size
NIB = NL // IB            # 4 i-blocks
NJT = M // 128            # 64 j-tiles
JG = 3                    # j-tiles per psum group (3 banks)
NJG = (NJT + JG - 1) // JG  # 22 groups (21x3 + 1x1)

EPC = 251904              # padded edges per core = 128*1968
EB = EPC // 128           # 1968 per partition
NCH = 8                   # edge chunks
CB = EB // NCH            # 246 edge-blocks per partition per chunk

_CACHE = {}


def _build_nc():
    nc = bacc.Bacc("TRN2", target_bir_lowering=False, debug=False,
                   num_devices=NCORES)
    with tile.TileContext(nc) as tc, ExitStack() as ctx:
        # ---------------- DRAM I/O ----------------
        z_loc = nc.dram_tensor("z_loc", [NL, D], F32, kind="ExternalInput")[:]
        gr_loc = nc.dram_tensor("gr_loc", [NL], F32, kind="ExternalInput")[:]
        w_full = nc.dram_tensor("w_full", [M, D], F32, kind="ExternalInput")[:]
        gc_tmaj = nc.dram_tensor("gc_tmaj", [128, NJT], F32, kind="ExternalInput")[:]
        ezr = nc.dram_tensor("ezr", [EPC, D], BF16, kind="ExternalInput")[:]
        ewr = nc.dram_tensor("ewr", [EPC, D], BF16, kind="ExternalInput")[:]
        egr = nc.dram_tensor("egr", [EPC], F32, kind="ExternalInput")[:]
        egc = nc.dram_tensor("egc", [EPC], F32, kind="ExternalInput")[:]
        ewt = nc.dram_tensor("ewt", [EPC], F32, kind="ExternalInput")[:]
        out_d = nc.dram_tensor("out", [1, 1], F32, kind="ExternalOutput")[:]
        eout_d = nc.dram_tensor("eout", [128, NCH], F32, kind="ExternalOutput")[:]
        dpool = None  # set below inside TileContext pools

        # ---------------- pools ----------------
        persist = ctx.enter_context(tc.tile_pool(name="persist", bufs=1))
        dram_pool = ctx.enter_context(tc.tile_pool(name="dram", bufs=1, space="DRAM"))
        ztb_t = dram_pool.tile([NL, 128], BF16, name="ztb_t")
        wtb_t = dram_pool.tile([M, 128], BF16, name="wtb_t")
        ztb_d = ztb_t[:]
        wtb_d = wtb_t[:]
        psq_pool = ctx.enter_context(tc.tile_pool(name="psq", bufs=2, space="PSUM"))
        acc_pool = ctx.enter_context(tc.tile_pool(name="acc", bufs=2, space="PSUM"))

        zb = persist.tile([128, 1], F32)
        nc.vector.memset(zb[:], 0.0)
        eb_ = persist.tile([128, 1], F32)
        nc.vector.memset(eb_[:], -EPS)

        # persistent result tiles
        er_sb = persist.tile([1, NL], F32)          # exp(gr_loc)
        ec_sb = persist.tile([128, NJT], BF16)      # exp(gc) per (p, jt)
        colsum = persist.tile([1, NL], F32)
        eacc = persist.tile([128, NCH], F32)        # per-chunk edge partials
        wT = persist.tile([128, M], BF16)           # operand rows 0..9 used
        zT = persist.tile([128, NL], BF16)

        # ============ phase 0: tables (exp set) ============
        with tc.tile_pool(name="ph0", bufs=1) as ph0:
            # w side softmax -> wtb
            wl = ph0.tile([128, M // 128, D], F32)
            nc.sync.dma_start(out=wl[:], in_=w_full.rearrange("(p b) d -> p b d", p=128))
            ewl = ph0.tile([128, M // 128, D], F32)
            nc.scalar.activation(ewl[:], wl[:], AF.Exp, bias=zb[:])
            swl = ph0.tile([128, M // 128], F32)
            nc.vector.tensor_reduce(swl[:], ewl[:], AX.X, ALU.add)
            rwl = ph0.tile([128, M // 128], F32)
            nc.vector.reciprocal(rwl[:], swl[:])
            rwl_b = bass.AP(rwl.tensor, rwl[:].offset, [*rwl[:].ap, [0, D]])
            wn = ph0.tile([128, M // 128, D], F32)
            nc.vector.tensor_tensor(wn[:], ewl[:], rwl_b, ALU.mult)
            w2t = ph0.tile([128, M // 128, D], F32)
            nc.vector.tensor_tensor(w2t[:], wn[:], wn[:], ALU.mult)
            w2 = ph0.tile([128, M // 128], F32)
            nc.vector.tensor_reduce(w2[:], w2t[:], AX.X, ALU.add)
            wtb = ph0.tile([128, M // 128, 128], BF16)
            nc.vector.memset(wtb[:], 0.0)
            nc.vector.tensor_copy(wtb[:, :, 0:D], wn[:])
            nc.vector.tensor_copy(wtb[:, :, D : D + 1], w2[:].rearrange("p (b o) -> p b o", o=1))
            nc.vector.memset(wtb[:, :, D + 1 : D + 2], 1.0)
            nc.sync.dma_start(out=wtb_d.rearrange("(p b) c -> p b c", p=128), in_=wtb[:])
            nc.sync.dma_start_transpose(out=wT[:], in_=wtb_d)

            # z side softmax -> ztb (scaled by -2)
            zl = ph0.tile([128, NL // 128, D], F32)
            nc.sync.dma_start(out=zl[:], in_=z_loc.rearrange("(p b) d -> p b d", p=128))
            ezl = ph0.tile([128, NL // 128, D], F32)
            nc.scalar.activation(ezl[:], zl[:], AF.Exp, bias=zb[:])
            szl = ph0.tile([128, NL // 128], F32)
            nc.vector.tensor_reduce(szl[:], ezl[:], AX.X, ALU.add)
            rzl = ph0.tile([128, NL // 128], F32)
            nc.vector.reciprocal(rzl[:], szl[:])
            rzl_b = bass.AP(rzl.tensor, rzl[:].offset, [*rzl[:].ap, [0, D]])
            zn = ph0.tile([128, NL // 128, D], F32)
            nc.vector.tensor_tensor(zn[:], ezl[:], rzl_b, ALU.mult)
            z2t = ph0.tile([128, NL // 128, D], F32)
            nc.vector.tensor_tensor(z2t[:], zn[:], zn[:], ALU.mult)
            z2 = ph0.tile([128, NL // 128], F32)
            nc.vector.tensor_reduce(z2[:], z2t[:], AX.X, ALU.add)
            ztb = ph0.tile([128, NL // 128, 128], BF16)
            nc.vector.memset(ztb[:], 0.0)
            nc.vector.tensor_scalar(ztb[:, :, 0:D], zn[:], -2.0, None, ALU.mult)
            nc.vector.memset(ztb[:, :, D : D + 1], 1.0)
            nc.vector.tensor_copy(ztb[:, :, D + 1 : D + 2], z2[:].rearrange("p (b o) -> p b o", o=1))
            nc.sync.dma_start(out=ztb_d.rearrange("(p b) c -> p b c", p=128), in_=ztb[:])
            nc.sync.dma_start_transpose(out=zT[:], in_=ztb_d)

            # gammas
            grt = ph0.tile([1, NL], F32)
            nc.sync.dma_start(out=grt[:], in_=gr_loc.rearrange("(p n) -> p n", p=1))
            nc.scalar.activation(er_sb[:], grt[:], AF.Exp, bias=zb[0:1])
            gct = ph0.tile([128, NJT], F32)
            nc.sync.dma_start(out=gct[:], in_=gc_tmaj)
            nc.scalar.activation(ec_sb[:], gct[:], AF.Exp, bias=zb[:])

        dist_pool = ctx.enter_context(tc.tile_pool(name="dist", bufs=2))
        epool = ctx.enter_context(tc.tile_pool(name="epool", bufs=1))
        esm = ctx.enter_context(tc.tile_pool(name="esm", bufs=2))

        # ============ edge chunk helpers ============
        echunks = [None] * NCH

        def edge_prep(g):
            """ACT exp (exp set) + DVE math for chunk g -> staged sq/gg/wt."""
            s = slice(g * CB, (g + 1) * CB)
            zr = epool.tile([128, CB, D], BF16, tag="zr")
            nc.sync.dma_start(out=zr[:], in_=ezr.rearrange("(p b) d -> p b d", p=128)[:, s, :])
            wr = epool.tile([128, CB, D], BF16, tag="wr")
            nc.sync.dma_start(out=wr[:], in_=ewr.rearrange("(p b) d -> p b d", p=128)[:, s, :])
            wtt = esm.tile([128, CB], F32, tag="wtt")
            nc.sync.dma_start(out=wtt[:], in_=ewt.rearrange("(p b) -> p b", p=128)[:, s])
            grt = esm.tile([128, CB], F32, tag="grt")
            nc.sync.dma_start(out=grt[:], in_=egr.rearrange("(p b) -> p b", p=128)[:, s])
            gct = esm.tile([128, CB], F32, tag="gct")
            nc.sync.dma_start(out=gct[:], in_=egc.rearrange("(p b) -> p b", p=128)[:, s])

            ez = zr
            nc.scalar.activation(ez[:], zr[:], AF.Exp, bias=zb[:])
            ew = wr
            nc.scalar.activation(ew[:], wr[:], AF.Exp, bias=zb[:])

            sz = esm.tile([128, CB], F32, tag="sz")
            nc.vector.tensor_reduce(sz[:], ez[:], AX.X, ALU.add)
            sw = esm.tile([128, CB], F32, tag="sw")
            nc.vector.tensor_reduce(sw[:], ew[:], AX.X, ALU.add)
            rz = esm.tile([128, CB], F32, tag="rz")
            nc.vector.reciprocal(rz[:], sz[:])
            rw = esm.tile([128, CB], F32, tag="rw")
            nc.vector.reciprocal(rw[:], sw[:])

            t8 = epool.tile([128, CB, D], BF16, tag="t8")
            red = esm.tile([128, CB], F32, tag="red")
            sq = esm.tile([128, CB], F32, tag="sq")
            tmp = esm.tile([128, CB], F32, tag="tmp")
            # zz/sz^2
            nc.vector.tensor_tensor(t8[:], ez[:], ez[:], ALU.mult)
            nc.vector.tensor_reduce(red[:], t8[:], AX.X, ALU.add)
            nc.vector.tensor_tensor(tmp[:], rz[:], rz[:], ALU.mult)
            nc.vector.tensor_tensor(sq[:], red[:], tmp[:], ALU.mult)
            # + ww/sw^2
            nc.vector.tensor_tensor(t8[:], ew[:], ew[:], ALU.mult)
            nc.vector.tensor_reduce(red[:], t8[:], AX.X, ALU.add)
            nc.vector.tensor_tensor(tmp[:], rw[:], rw[:], ALU.mult)
            nc.vector.tensor_tensor(tmp[:], red[:], tmp[:], ALU.mult)
            nc.vector.tensor_tensor(sq[:], sq[:], tmp[:], ALU.add)
            # - 2 zw/(sz*sw)
            nc.vector.tensor_tensor(t8[:], ez[:], ew[:], ALU.mult)
            nc.vector.tensor_reduce(red[:], t8[:], AX.X, ALU.add)
            nc.vector.tensor_tensor(tmp[:], rz[:], rw[:], ALU.mult)
            nc.vector.tensor_tensor(tmp[:], red[:], tmp[:], ALU.mult)
            nc.vector.tensor_tensor(sq[:], sq[:], tmp[:], ALU.subtract)
            nc.vector.tensor_tensor(sq[:], sq[:], tmp[:], ALU.subtract)
            nc.vector.tensor_scalar(sq[:], sq[:], 0.0, None, ALU.max)
            # gg = gr_e + gc_e  (in place over grt)
            nc.vector.tensor_tensor(grt[:], grt[:], gct[:], ALU.add)
            echunks[g] = (sq, grt, wtt)

        def edge_finish(g):
            """ACT sqrt (sqrt set) + weighted accumulation for chunk g."""
            sq, gg, wtt = echunks[g]
            nc.scalar.activation(sq[:], sq[:], AF.Sqrt, bias=zb[:])
            nc.vector.tensor_tensor(gg[:], gg[:], sq[:], ALU.subtract)
            nc.vector.tensor_tensor(gg[:], gg[:], wtt[:], ALU.mult)
            nc.vector.tensor_reduce(eacc[:, g : g + 1], gg[:], AX.X, ALU.add)
            echunks[g] = None

        # ============ main loop ============
        for b in range(NIB):
            isl = slice(b * IB, (b + 1) * IB)
            dist_in = dist_pool.tile([128, NJT * IB], BF16, tag="dist")
            # PE aug-matmuls + DVE clamp, grouped by psum allocation
            for g in range(NJG):
                ntile = min(JG, NJT - g * JG)
                psq = psq_pool.tile([128, JG * IB], F32, tag="psq")
                for u in range(ntile):
                    jt = g * JG + u
                    nc.tensor.matmul(
                        psq[:, u * IB : (u + 1) * IB],
                        wT[0:10, jt * 128 : (jt + 1) * 128],
                        zT[0:10, isl],
                        start=True, stop=True)
                o = g * JG * IB
                nc.vector.tensor_scalar(
                    dist_in[:, o : o + ntile * IB], psq[:, 0 : ntile * IB],
                    0.0, None, ALU.max)
            # ACT sqrt phase
            nc.scalar.activation(dist_in[:], dist_in[:], AF.Sqrt, bias=zb[:])
            if b > 0:
                edge_finish(2 * (b - 1))
                edge_finish(2 * (b - 1) + 1)
            # ACT exp phase
            nc.scalar.activation(dist_in[:], dist_in[:], AF.Exp, bias=eb_[:],
                                 scale=-1.0)
            edge_prep(2 * b)
            edge_prep(2 * b + 1)
            # PE reduce-matmuls: colsum[i] += sum_j ec_j val[j, i]
            acc = acc_pool.tile([1, IB], F32, tag="acc")
            for jt in range(NJT):
                nc.tensor.matmul(
                    acc[:],
                    ec_sb[:, jt : jt + 1],
                    dist_in[:, jt * IB : (jt + 1) * IB],
                    start=(jt == 0), stop=(jt == NJT - 1))
            nc.vector.tensor_copy(colsum[:, isl], acc[:])

        # ============ tail ============
        edge_finish(NCH - 2)
        edge_finish(NCH - 1)

        nc.vector.tensor_tensor(colsum[:], colsum[:], er_sb[:], ALU.mult)
        z1 = persist.tile([1, 1], F32)
        nc.vector.tensor_reduce(z1[:], colsum[:], AX.X, ALU.add)
        nc.sync.dma_start(out=out_d, in_=z1[:])
        nc.sync.dma_start(out=eout_d, in_=eacc[:])
    nc.compile()
    return nc


def _prep_inputs(gamma_rows, gamma_cols, latent_z, latent_w, weights,
                 rows_idx, col_idx):
    gamma_rows = np.asarray(gamma_rows, dtype=np.float32)
    gamma_cols = np.asarray(gamma_cols, dtype=np.float32)
    latent_z = np.asarray(latent_z, dtype=np.float32)
    latent_w = np.asarray(latent_w, dtype=np.float32)
    weights = np.asarray(weights, dtype=np.float32)
    rows_idx = np.asarray(rows_idx, dtype=np.int32)
    col_idx = np.asarray(col_idx, dtype=np.int32)

    gc_tmaj = np.ascontiguousarray(gamma_cols.reshape(NJT, 128).T)
    epc_raw = E // NCORES
    in_maps = []
    for c in range(NCORES):
        es = slice(c * epc_raw, (c + 1) * epc_raw)
        ri = rows_idx[es]
        ci = col_idx[es]
        pad = EPC - epc_raw
        ri_p = np.concatenate([ri, np.zeros(pad, np.int32)])
        ci_p = np.concatenate([ci, np.zeros(pad, np.int32)])
        wt_p = np.concatenate([weights[es], np.zeros(pad, np.float32)])
        in_maps.append({
            "z_loc": latent_z[c * NL : (c + 1) * NL],
            "gr_loc": gamma_rows[c * NL : (c + 1) * NL],
            "w_full": latent_w,
            "gc_tmaj": gc_tmaj,
            "ezr": latent_z[ri_p].astype(ml_dtypes.bfloat16),
            "ewr": latent_w[ci_p].astype(ml_dtypes.bfloat16),
            "egr": gamma_rows[ri_p],
            "egc": gamma_cols[ci_p],
            "ewt": wt_p,
        })
    return in_maps


def kernel(gamma_rows, gamma_cols, latent_z, latent_w, weights,
           rows_idx, col_idx, _trace=False, _trace_kwargs=None):
    if "nc" not in _CACHE:
        _CACHE["nc"] = _build_nc()
    nc = _CACHE["nc"]
    in_maps = _prep_inputs(gamma_rows, gamma_cols, latent_z, latent_w,
                           weights, rows_idx, col_idx)
    kw = {}
    if _trace:
        kw = {"trace": True, **(_trace_kwargs or {})}
    res = run_bass_kernel_spmd(nc, in_maps, list(range(NCORES)), **kw)
    total = np.float64(0.0)
    for r in res.results:
        total += np.float64(r["out"][0, 0]) - np.float64(r["eout"].sum(dtype=np.float64))
    out = np.float32(total)
    if _trace:
        _CACHE["last_result"] = res
    return np.asarray(out)

